# revision 5
# baseline (speedup 1.0000x reference)
"""BERT encoder (B=16, S=512, D=768, H=12, L=4, FF=3072) on 8 trn2 NeuronCores.

Sharding: data-parallel over batch -> 2 sequences per core. No collectives.
Per-core kernel: feature-major activations xT [768, 1024], fp32r matmuls.
"""

import numpy as np

from contextlib import ExitStack

import concourse.bass as bass
import concourse.mybir as mybir
import concourse.tile as tile
from concourse import bacc
from concourse.bass_utils import run_bass_kernel_spmd

F32 = mybir.dt.float32
F32R = mybir.dt.float32r
AF = mybir.ActivationFunctionType
ALU = mybir.AluOpType

N_CORES = 8
B, S, D, H, L, FF, V = 16, 512, 768, 12, 4, 3072, 30522
DH = D // H            # 64
T = 1024               # tokens per core (2 seqs x 512)
KD = D // 128          # 6 k-chunks over D
KF = FF // 128         # 24 chunks over FF
NSEQ = 2               # seqs per core
NEG = -30000.0         # additive mask (exp underflows to 0)

_cache = {}


def _seq_cols(s):
    return slice(512 * s, 512 * s + 512)


def _build(use_beta, use_embb):
    nc = bacc.Bacc("TRN2", target_bir_lowering=False, debug=False,
                   num_devices=N_CORES)

    dp = nc.declare_dram_parameter
    embT_in = dp("embT", [KD, 128, T], F32, isOutput=False)
    wqk_in = dp("wqk", [L, KD, 128, KD, 256], F32, isOutput=False)
    wv_in = dp("wv", [L, KD, 128, D], F32, isOutput=False)
    wo_in = dp("wo", [L, KD, 128, KD, 128], F32, isOutput=False)
    w1_in = dp("w1", [L, KF, 128, KD, 128], F32, isOutput=False)
    w2_in = dp("w2", [L, KF, 128, D], F32, isOutput=False)
    # bias8 cols: 0 bq, 1 bk, 2 c'(=bo+bv@Wo), 3 b2, 4 ln_g, 5 ln_b
    bias8_in = dp("bias8", [L, 128, KD, 8], F32, isOutput=False)
    b1_in = dp("b1", [L, 128, KF], F32, isOutput=False)
    embgb_in = dp("embgb", [128, KD, 2], F32, isOutput=False)
    wf_in = dp("wf", [128, KD, 2], F32, isOutput=False)
    bf_in = dp("bf", [2, 2], F32, isOutput=False)
    mb_in = dp("mb", [128, 2 * NSEQ * 4], F32, isOutput=False)  # unused cols ok
    consts_in = dp("consts", [128, 2], F32, isOutput=False)     # 1.0, 1/768
    out_par = dp("out", [NSEQ, 2], F32, isOutput=True)

    with tile.TileContext(nc) as tc, nc.allow_low_precision(reason="fp32r"):
        # ---------------- persistent pools ----------------
        _acts_ctx = ExitStack()
        acts = _acts_ctx.enter_context(tc.tile_pool(name="acts", bufs=1))
        # x-tag: main activation sets rotate (x_layer, xA, x_next)
        def new_x(tag):
            return [acts.tile([128, T], F32R, tag=f"{tag}{j}", bufs=2,
                              name=f"{tag}{j}")
                    for j in range(KD)]

        consts = acts.tile([128, 2], F32R, tag="consts", bufs=1)
        nc.sync.dma_start(out=consts, in_=consts_in[:, :].bitcast(F32R))
        ones_col = consts[:, 0:1]
        onesD_col = consts[:, 1:2]
        mb_t = acts.tile([128, 2 * NSEQ * 4], F32, tag="mb", bufs=1)
        nc.sync.dma_start(out=mb_t, in_=mb_in[:, :])
        eps_t = acts.tile([128, 2], F32, tag="eps", bufs=1)
        nc.vector.memset(eps_t[:, 0:1], 1e-12)
        nc.vector.memset(eps_t[:, 1:2], 1e-6)

        Vt = [acts.tile([128, D], F32R, tag=f"V{t}", bufs=1, name=f"V{t}") for t in range(8)]
        ON = [acts.tile([128, T], F32R, tag=f"ON{j}", bufs=1, name=f"ON{j}") for j in range(KD)]

        # ---------------- helpers ----------------
        def layernorm(xin, xout, g_col, b_col, eps_col, has_beta):
            """xout[j] = LN(xin[j]) over feature dim (partitions across 6 tiles).

            g_col/b_col: [128,1] AP per j -> callables j -> AP.
            """
            ln_ctx = ExitStack()
            scratch = ln_ctx.enter_context(tc.tile_pool(name="p_ln", bufs=1))
            psum = ln_ctx.enter_context(
                tc.tile_pool(name="ps_ln", bufs=1, space="PSUM"))
            for s in range(NSEQ):
                cols = _seq_cols(s)
                ps_m = psum.tile([1, 512], F32, tag="stat_m", bufs=2)
                ps_e = psum.tile([1, 512], F32, tag="stat_e", bufs=2)
                for j in range(KD):
                    sq = scratch.tile([128, 512], F32R, tag="sq", bufs=2)
                    nc.scalar.activation(out=sq, in_=xin[j][:, cols],
                                         func=AF.Square, bias=0.0, scale=1.0)
                    nc.tensor.matmul(ps_m, onesD_col, xin[j][:, cols],
                                     start=(j == 0), stop=(j == KD - 1))
                    nc.tensor.matmul(ps_e, onesD_col, sq,
                                     start=(j == 0), stop=(j == KD - 1))
                ms = scratch.tile([1, 512], F32, tag="ms", bufs=2)
                nc.vector.tensor_copy(out=ms, in_=ps_m)
                m2 = scratch.tile([1, 512], F32, tag="m2", bufs=2)
                nc.vector.tensor_tensor(out=m2, in0=ms, in1=ms, op=ALU.mult)
                vs = scratch.tile([1, 512], F32, tag="vs", bufs=2)
                nc.vector.tensor_tensor(out=vs, in0=ps_e, in1=m2, op=ALU.subtract)
                sd = scratch.tile([1, 512], F32, tag="sd", bufs=2)
                nc.scalar.activation(out=sd, in_=vs, func=AF.Sqrt,
                                     bias=eps_col, scale=1.0)
                rs = scratch.tile([1, 512], F32, tag="rs", bufs=2)
                nc.vector.reciprocal(out=rs, in_=sd)
                mbb = scratch.tile([128, 512], F32, tag="mb_b", bufs=2)
                nc.gpsimd.partition_broadcast(mbb, ms)
                rbb = scratch.tile([128, 512], F32, tag="rb_b", bufs=2)
                nc.gpsimd.partition_broadcast(rbb, rs)
                for j in range(KD):
                    u1 = scratch.tile([128, 512], F32R, tag="u1", bufs=2)
                    nc.vector.tensor_tensor(out=u1, in0=xin[j][:, cols],
                                            in1=mbb.bitcast(F32R), op=ALU.subtract)
                    nc.vector.scalar_tensor_tensor(
                        out=xout[j][:, cols], in0=u1, scalar=g_col(j),
                        in1=rbb.bitcast(F32R), op0=ALU.mult, op1=ALU.mult)
                    if has_beta:
                        nc.vector.tensor_scalar(
                            out=xout[j][:, cols], in0=xout[j][:, cols],
                            scalar1=b_col(j), scalar2=None, op0=ALU.add)
            ln_ctx.close()

        # ---------------- embedding LN ----------------
        x = new_x("x0")  # LN(emb)
        embgb2 = acts.tile([128, KD, 2], F32, tag="embgb", bufs=1)
        nc.sync.dma_start(out=embgb2, in_=embgb_in[:, :, :])
        x_pre_emb = new_x("u")  # reuse u-tag slots for raw embeddings
        for j in range(KD):
            nc.sync.dma_start(out=x_pre_emb[j], in_=embT_in[j].bitcast(F32R))
        layernorm(x_pre_emb, x, lambda j: embgb2[:, j, 0:1],
                  lambda j: embgb2[:, j, 1:2], eps_t[0:1, 0:1], use_embb)

        # ---------------- layers ----------------
        for l in range(L):
            bias8 = acts.tile([128, KD, 8], F32, tag="bias8", bufs=2)
            nc.sync.dma_start(out=bias8, in_=bias8_in[l])
            b1t = acts.tile([128, KF], F32, tag="b1t", bufs=2)
            nc.sync.dma_start(out=b1t, in_=b1_in[l])

            # ---- V projection: V[t] [128 tok, 768] token-major ----
            with tc.tile_pool(name="p_v", bufs=1) as p_v, \
                 tc.tile_pool(name="ps_v", bufs=1, space="PSUM") as ps_v:
                wv_t = [p_v.tile([128, D], F32R, tag=f"wv{k}", bufs=1,
                                 name=f"wv{k}")
                        for k in range(KD)]
                for k in range(KD):
                    nc.sync.dma_start(out=wv_t[k], in_=wv_in[l, k].bitcast(F32R))
                # V: out[tok, dcol] = sum_din x[din, tok] * wv[din, dcol]
                #   lhsT = x[k][:, tok-chunk] (K=din, M=tok), rhs = wv[k][:, cols]
                for t in range(8):
                    for half in range(2):
                        ncol = slice(384 * half, 384 * half + 384)
                        ps = ps_v.tile([128, 512], F32, tag="pv", bufs=3)
                        for k in range(KD):
                            nc.tensor.matmul(
                                ps[:, 0:384],
                                x[k][:, 128 * t:128 * t + 128],
                                wv_t[k][:, ncol],
                                start=(k == 0), stop=(k == KD - 1))
                        nc.vector.tensor_copy(out=Vt[t][:, ncol], in_=ps[:, 0:384])

            # ---- attention per head-pair ----
            with tc.tile_pool(name="p_at", bufs=1) as p_at, \
                 tc.tile_pool(name="ps_at", bufs=1, space="PSUM") as ps_at:
                for jp in range(KD):
                    wqk_t = p_at.tile([128, KD, 256], F32R, tag="wqk", bufs=2)
                    nc.sync.dma_start(out=wqk_t, in_=wqk_in[l, jp].bitcast(F32R))
                    for s in range(NSEQ):
                        cols = _seq_cols(s)
                        qkt = p_at.tile([128, 2, 512], F32R, tag="qkt", bufs=2)
                        for qk in range(2):
                            ps = ps_at.tile([128, 512], F32, tag="proj", bufs=2)
                            for k in range(KD):
                                nc.tensor.matmul(
                                    ps, wqk_t[:, k, 128 * qk:128 * qk + 128],
                                    x[k][:, cols],
                                    start=(k == 0), stop=(k == KD - 1))
                            nc.scalar.activation(
                                out=qkt[:, qk], in_=ps, func=AF.Identity,
                                bias=bias8[:, jp, qk:qk + 1], scale=1.0)
                        for hs in range(2):
                            rows = slice(64 * hs, 64 * hs + 64)
                            dcol = 128 * jp + 64 * hs  # head dim offset in V
                            ps_o = ps_at.tile([128, 512], F32, tag="av", bufs=2)
                            ps_z = ps_at.tile([1, 512], F32, tag="z", bufs=2)
                            for c in range(4):
                                ps_s = ps_at.tile([128, 512], F32, tag="sc", bufs=2)
                                nc.tensor.matmul(
                                    ps_s,
                                    qkt[rows, 1, 128 * c:128 * c + 128],
                                    qkt[rows, 0, :],
                                    start=True, stop=True)
                                ae = p_at.tile([128, 512], F32R, tag="ae", bufs=2)
                                nc.scalar.activation(
                                    out=ae, in_=ps_s, func=AF.Exp,
                                    bias=mb_t[:, 4 * s + c:4 * s + c + 1],
                                    scale=0.125)
                                nc.tensor.matmul(
                                    ps_o[0:64, :], Vt[4 * s + c][:, dcol:dcol + 64],
                                    ae, start=(c == 0), stop=(c == 3))
                                nc.tensor.matmul(
                                    ps_z, ones_col, ae,
                                    start=(c == 0), stop=(c == 3))
                            zr = p_at.tile([1, 512], F32, tag="zr", bufs=2)
                            nc.vector.reciprocal(out=zr, in_=ps_z)
                            zb = p_at.tile([64, 512], F32, tag="zb", bufs=2)
                            nc.gpsimd.partition_broadcast(zb, zr)
                            nc.vector.scalar_tensor_tensor(
                                out=ON[jp][rows, cols], in0=ps_o[0:64, :],
                                scalar=1.0, in1=zb.bitcast(F32R),
                                op0=ALU.mult, op1=ALU.mult)

            # ---- Wo projection + residual + LN -> xA ----
            u = new_x(f"u")
            with tc.tile_pool(name="p_wo", bufs=1) as p_wo, \
                 tc.tile_pool(name="ps_wo", bufs=1, space="PSUM") as ps_wo:
                for j in range(KD):
                    wo_t = p_wo.tile([128, KD, 128], F32R, tag="wo", bufs=2)
                    nc.sync.dma_start(out=wo_t, in_=wo_in[l, j].bitcast(F32R))
                    for s in range(NSEQ):
                        cols = _seq_cols(s)
                        ps = ps_wo.tile([128, 512], F32, tag="proj", bufs=3)
                        for k in range(KD):
                            nc.tensor.matmul(ps, wo_t[:, k], ON[k][:, cols],
                                             start=(k == 0), stop=(k == KD - 1))
                        nc.vector.scalar_tensor_tensor(
                            out=u[j][:, cols], in0=ps,
                            scalar=bias8[:, j, 2:3], in1=x[j][:, cols],
                            op0=ALU.add, op1=ALU.add)
            xA = new_x("x0")
            layernorm(u, xA, lambda j: bias8[:, j, 4:5],
                      lambda j: bias8[:, j, 5:6], eps_t[0:1, 1:2], use_beta)

            # ---- FFN + residual + LN -> x_next ----
            v = new_x("u")
            with tc.tile_pool(name="p_ff", bufs=1) as p_ff, \
                 tc.tile_pool(name="ps_ff", bufs=1, space="PSUM") as ps_ff:
                for s in range(NSEQ):
                    cols = _seq_cols(s)
                    ps_out = ps_ff.tile([128, KD, 512], F32, tag="ffn_out", bufs=1)
                    for f in range(KF):
                        w1_t = p_ff.tile([128, KD, 128], F32R, tag="w1", bufs=3)
                        nc.sync.dma_start(out=w1_t, in_=w1_in[l, f].bitcast(F32R))
                        w2_t = p_ff.tile([128, D], F32R, tag="w2", bufs=3)
                        nc.sync.dma_start(out=w2_t, in_=w2_in[l, f].bitcast(F32R))
                        ps_h = ps_ff.tile([128, 512], F32, tag="h", bufs=2)
                        for k in range(KD):
                            nc.tensor.matmul(ps_h, w1_t[:, k], xA[k][:, cols],
                                             start=(k == 0), stop=(k == KD - 1))
                        hf = p_ff.tile([128, 512], F32R, tag="hf", bufs=3)
                        nc.scalar.activation(out=hf, in_=ps_h, func=AF.Relu,
                                             bias=b1t[:, f:f + 1], scale=1.0)
                        for j in range(KD):
                            nc.tensor.matmul(
                                ps_out[:, j], w2_t[:, 128 * j:128 * j + 128],
                                hf, start=(f == 0), stop=(f == KF - 1))
                    for j in range(KD):
                        nc.vector.scalar_tensor_tensor(
                            out=v[j][:, cols], in0=ps_out[:, j],
                            scalar=bias8[:, j, 3:4], in1=xA[j][:, cols],
                            op0=ALU.add, op1=ALU.add)
            x_next = new_x("x0")
            layernorm(v, x_next, lambda j: bias8[:, j, 4:5],
                      lambda j: bias8[:, j, 5:6], eps_t[0:1, 1:2], use_beta)
            x = x_next

        # ---------------- classifier on CLS tokens ----------------
        with tc.tile_pool(name="p_cls", bufs=1) as p_cls, \
             tc.tile_pool(name="ps_cls", bufs=1, space="PSUM") as ps_cls:
            wf_t = p_cls.tile([128, KD, 2], F32R, tag="wf", bufs=1)
            nc.sync.dma_start(out=wf_t, in_=wf_in[:, :, :].bitcast(F32R))
            bf_t = p_cls.tile([2, 2], F32, tag="bf", bufs=1)
            nc.sync.dma_start(out=bf_t, in_=bf_in[:, :])
            ps = ps_cls.tile([2, 2], F32, tag="cls", bufs=1)
            for j in range(KD):
                # lhsT = x[j][:, {0, 512}] -> [128, 2]; rhs = wf[j] [128, 2]
                nc.tensor.matmul(ps, x[j][:, 0:1024:512], wf_t[:, j],
                                 start=(j == 0), stop=(j == KD - 1))
            lg = p_cls.tile([2, 2], F32, tag="lg", bufs=1)
            nc.vector.tensor_tensor(out=lg, in0=ps, in1=bf_t, op=ALU.add)
            eg = p_cls.tile([2, 2], F32, tag="eg", bufs=1)
            ssum = p_cls.tile([2, 1], F32, tag="ss", bufs=1)
            nc.scalar.activation(out=eg, in_=lg, func=AF.Exp, bias=0.0,
                                 scale=1.0, accum_out=ssum)
            si = p_cls.tile([2, 1], F32, tag="si", bufs=1)
            nc.vector.reciprocal(out=si, in_=ssum)
            pr = p_cls.tile([2, 2], F32, tag="pr", bufs=1)
            nc.vector.tensor_scalar(out=pr, in0=eg, scalar1=si, scalar2=None,
                                    op0=ALU.mult)
            nc.sync.dma_start(out=out_par[:, :], in_=pr)

        _acts_ctx.close()

    nc.compile()
    return nc


def _pack_host(params):
    p = {k: np.asarray(v, dtype=np.float32) if np.asarray(v).dtype != np.int32
         else np.asarray(v) for k, v in params.items()}
    Wq, Wk, Wv, Wo = p['Wq'], p['Wk'], p['Wv'], p['Wo']
    W1, W2 = p['W1'], p['W2']

    def blk(W):  # [L, 768, 768] -> [L, jp(out), 128(d_in), k(in), 128(col)]
        return W.reshape(L, KD, 128, KD, 128).transpose(0, 3, 2, 1, 4)

    wqk = np.ascontiguousarray(np.concatenate([blk(Wq), blk(Wk)], axis=-1))
    wv = np.ascontiguousarray(Wv.reshape(L, KD, 128, D))
    wo = np.ascontiguousarray(blk(Wo))
    w1 = np.ascontiguousarray(
        W1.reshape(L, KD, 128, KF, 128).transpose(0, 3, 2, 1, 4))
    w2 = np.ascontiguousarray(W2.reshape(L, KF, 128, D))

    def col6(vec):  # [L?, 768] -> [..., 128, 6]
        return np.ascontiguousarray(vec.reshape(-1, KD, 128).transpose(0, 2, 1))

    cprime = np.einsum('ld,ldo->lo', p['bv'], Wo) + p['bo']  # [L, 768]
    bias8 = np.zeros((L, 128, KD, 8), np.float32)
    bias8[..., 0] = col6(p['bq'])
    bias8[..., 1] = col6(p['bk'])
    bias8[..., 2] = col6(cprime)
    bias8[..., 3] = col6(p['b2'])
    bias8[..., 4] = col6(p['ln_g'])
    bias8[..., 5] = col6(p['ln_b'])
    b1 = np.ascontiguousarray(p['b1'].reshape(L, KF, 128).transpose(0, 2, 1))
    embgb = np.stack([p['emb_g'].reshape(KD, 128).T,
                      p['emb_b'].reshape(KD, 128).T], axis=-1)
    embgb = np.ascontiguousarray(embgb)
    wf = np.ascontiguousarray(p['Wf'].reshape(KD, 128, 2).transpose(1, 0, 2))
    bf = np.broadcast_to(p['bf'], (2, 2)).astype(np.float32).copy()
    consts = np.zeros((128, 2), np.float32)
    consts[:, 0] = 1.0
    consts[:, 1] = 1.0 / D
    use_beta = bool(np.any(p['ln_b'] != 0))
    use_embb = bool(np.any(p['emb_b'] != 0))
    shared = dict(wqk=wqk, wv=wv, wo=wo, w1=w1, w2=w2, bias8=bias8, b1=b1,
                  embgb=embgb, wf=wf, bf=bf, consts=consts)
    return p, shared, use_beta, use_embb


def kernel(params, input_ids, token_type_ids, mask):
    input_ids = np.asarray(input_ids)
    token_type_ids = np.asarray(token_type_ids)
    mask = np.asarray(mask)
    p, shared, use_beta, use_embb = _pack_host(params)

    key = (use_beta, use_embb)
    if key not in _cache:
        _cache[key] = _build(use_beta, use_embb)
    nc = _cache[key]

    # embeddings on host (gather + adds), then feature-major transpose
    emb = p['word_emb'][input_ids] + p['pos_emb'][None, :, :] \
        + p['tok_emb'][token_type_ids]          # [B, S, D] f32
    count = (mask != 0).sum(axis=-1)            # [B]
    keep = np.arange(S)[None, :] < count[:, None]   # [B, S]

    in_maps = []
    for c in range(N_CORES):
        e = emb[2 * c:2 * c + 2].reshape(T, D)  # [1024, 768]
        embT = np.ascontiguousarray(e.T).reshape(KD, 128, T)
        mb = np.zeros((128, 2 * NSEQ * 4), np.float32)
        for s in range(NSEQ):
            kp = keep[2 * c + s].reshape(4, 128)  # [chunk, partition]
            for ch in range(4):
                mb[:, 4 * s + ch] = np.where(kp[ch], 0.0, NEG)
        m = dict(shared)
        m['embT'] = embT
        m['mb'] = mb
        in_maps.append(m)

    res = run_bass_kernel_spmd(nc, in_maps, list(range(N_CORES)))
    out = np.concatenate([res.results[c]['out'] for c in range(N_CORES)], axis=0)
    return out.astype(np.float32)
